# revision 4
# baseline (speedup 1.0000x reference)
"""MultiHeadAttention Trainium2 kernel (8-core SPMD, no collectives).

Problem: B=4, T=2048, E=1024, H=16, D=64 multi-head self-attention
(torch-style Linear projections, softmax over keys, output projection).

Sharding: core c handles batch b=c//2 and query-half qh=c%2 (1024 query
tokens).  K/V are recomputed locally over the full 2048 tokens of the
batch element, so no cross-core communication is needed.  Inputs are
prepared host-side per core (transposed / bf16-cast / permuted so the
core's own query half occupies the first 1024 token columns -- softmax
over keys is permutation invariant); outputs are concatenated host-side.

Device pipeline per core (storage bf16, all accumulation fp32):
  - qT/kT projections (feature-major: out [feature, token]),
  - v projection (token-major) written into "vaug" [v_h | 1] blocks,
  - per head: scores^T = kT_h^T q_h via K=64 matmuls, row-tiled
    tile_position (0,0)/(64,0) so the two heads of a feature chunk run on
    independent 64x128 PE tiles,
  - exp on ScalarE (scale=1/8 fused; no max subtraction -- logits are
    bounded by construction),
  - PV matmul with lhsT=[v|1]: row 64 of the psum accumulates the softmax
    denominator for free,
  - K=1 fp32 ones-matmul broadcasts the denominator row across all
    partitions; DVE reciprocal + multiply normalizes,
  - odd heads' outputs are partition-shifted 0:64 -> 64:128 with a tiny
    SBUF->SBUF DMA so outT keeps the natural feature-major layout,
  - output projection (lhsT = outT tiles) with bo added on evacuation.
"""

import os
import sys
from contextlib import ExitStack

import numpy as np
import ml_dtypes

for _p in ("/opt/trn_rl_repo", "/root/.axon_site/_ro/trn_rl_repo"):
    if os.path.isdir(_p) and _p not in sys.path:
        sys.path.insert(0, _p)

import concourse.bass as bass  # noqa: E402,F401
from concourse import bacc  # noqa: E402
import concourse.tile as tile  # noqa: E402
from concourse import mybir  # noqa: E402
from concourse.bass_utils import run_bass_kernel_spmd  # noqa: E402

# ---- problem constants (hardcoded; kernel.py must be self-contained) ----
B, T, E, H, D = 4, 2048, 1024, 16, 64
P = 128
NCORES = 8
QT = T // 2          # query tokens per core = 1024
EC = E // P          # 8   e-chunks (contraction chunks for projections)
FC = E // P          # 8   f-chunks (feature chunks = head pairs)
KC = T // P          # 16  key-token chunks
HPAIR = H // 2       # 8 head pairs

BF = mybir.dt.bfloat16
F32 = mybir.dt.float32
AF = mybir.ActivationFunctionType
ALU = mybir.AluOpType

SC_G = 2             # kc-chunks per exp ACT instruction
USE_TILE_POS = os.environ.get("KERNEL_NO_TILEPOS", "0") != "1"


def build_program():
    nc = bacc.Bacc("TRN2", target_bir_lowering=False, debug=False,
                   num_devices=NCORES)

    xt_d = nc.dram_tensor("xt", [EC, P, T], BF, kind="ExternalInput").ap()
    wqt_d = nc.dram_tensor("wqt", [EC, P, E], BF, kind="ExternalInput").ap()
    wkt_d = nc.dram_tensor("wkt", [EC, P, E], BF, kind="ExternalInput").ap()
    wvt_d = nc.dram_tensor("wvt", [EC, P, E], BF, kind="ExternalInput").ap()
    wot_d = nc.dram_tensor("wot", [FC, P, E], BF, kind="ExternalInput").ap()
    bq_d = nc.dram_tensor("bq", [FC, P], F32, kind="ExternalInput").ap()
    bk_d = nc.dram_tensor("bk", [FC, P], F32, kind="ExternalInput").ap()
    bvb_d = nc.dram_tensor("bvb", [P, E], F32, kind="ExternalInput").ap()
    bob_d = nc.dram_tensor("bob", [P, E], F32, kind="ExternalInput").ap()
    out_d = nc.dram_tensor("out", [QT // P, P, E], F32,
                           kind="ExternalOutput").ap()

    with tile.TileContext(nc) as tc, ExitStack() as ctx:
        persist = ctx.enter_context(tc.tile_pool(name="persist", bufs=1))
        wq_pool = ctx.enter_context(tc.tile_pool(name="wq", bufs=2))
        wv_pool = ctx.enter_context(tc.tile_pool(name="wv", bufs=1))
        exp_pool = ctx.enter_context(tc.tile_pool(name="expp", bufs=2))
        small = ctx.enter_context(tc.tile_pool(name="small", bufs=1))
        otmp_pool = ctx.enter_context(tc.tile_pool(name="otmp", bufs=1))
        fin_pool = ctx.enter_context(tc.tile_pool(name="finp", bufs=1))
        psc = ctx.enter_context(tc.tile_pool(name="psc", bufs=1, space="PSUM"))
        pmm = ctx.enter_context(tc.tile_pool(name="pmm", bufs=2, space="PSUM"))
        ppv = ctx.enter_context(tc.tile_pool(name="ppv", bufs=1, space="PSUM"))

        # ---------------- persistent SBUF tensors ----------------
        xt_sb = persist.tile([P, EC, T], BF, tag="xt")          # 32K
        bq_sb = persist.tile([P, FC], F32, tag="bq")
        bk_sb = persist.tile([P, FC], F32, tag="bk")
        bvb_sb = persist.tile([P, E], F32, tag="bvb")           # 4K
        bob_sb = persist.tile([P, E], F32, tag="bob")           # 4K
        wot_sb = persist.tile([P, FC, E], BF, tag="wot")        # 16K
        qt_sb = persist.tile([P, FC, QT], BF, tag="qt")         # 16K
        kt_sb = persist.tile([P, FC, T], BF, tag="kt")          # 32K
        vaug = persist.tile([P, KC, H * 65], BF, tag="vaug")    # 32.5K
        outT = persist.tile([P, FC, QT], BF, tag="outT")        # 16K
        ones_sb = persist.tile([P, 64], F32, tag="ones")

        # ---------------- input DMAs ----------------
        for ec in range(EC):
            nc.sync.dma_start(xt_sb[:, ec, :], xt_d[ec])
        nc.sync.dma_start(bq_sb[:], bq_d.rearrange("f p -> p f"))
        nc.sync.dma_start(bk_sb[:], bk_d.rearrange("f p -> p f"))
        nc.sync.dma_start(bvb_sb[:], bvb_d)

        # constants
        nc.gpsimd.memset(ones_sb[:], 1.0)
        vaug_h = vaug.rearrange("p k (h c) -> p k h c", c=65)
        for h in range(H):
            nc.gpsimd.memset(vaug_h[:, :, h, 64:65], 1.0)

        # ---------------- q / k projections (feature-major) ----------------
        # qT[f, t] = sum_e WqT[e, f] * xT[e, t] ; + bias on evacuation
        for fc in range(FC):
            fs = slice(fc * P, (fc + 1) * P)
            wqf = wq_pool.tile([P, EC, P], BF, tag="wqf")
            wkf = wq_pool.tile([P, EC, P], BF, tag="wkf")
            for ec in range(EC):
                nc.sync.dma_start(wqf[:, ec, :], wqt_d[ec, :, fs])
                nc.sync.dma_start(wkf[:, ec, :], wkt_d[ec, :, fs])
            for tb in range(QT // 512):
                ps = pmm.tile([P, 512], F32, tag="mm")
                for ec in range(EC):
                    nc.tensor.matmul(
                        ps[:], lhsT=wqf[:, ec, :],
                        rhs=xt_sb[:, ec, tb * 512:(tb + 1) * 512],
                        start=(ec == 0), stop=(ec == EC - 1),
                    )
                nc.vector.tensor_scalar_add(
                    qt_sb[:, fc, tb * 512:(tb + 1) * 512], ps[:],
                    bq_sb[:, fc: fc + 1],
                )
            for tb in range(T // 512):
                ps = pmm.tile([P, 512], F32, tag="mm")
                for ec in range(EC):
                    nc.tensor.matmul(
                        ps[:], lhsT=wkf[:, ec, :],
                        rhs=xt_sb[:, ec, tb * 512:(tb + 1) * 512],
                        start=(ec == 0), stop=(ec == EC - 1),
                    )
                nc.vector.tensor_scalar_add(
                    kt_sb[:, fc, tb * 512:(tb + 1) * 512], ps[:],
                    bk_sb[:, fc: fc + 1],
                )

        # ---------------- v projection (token-major, +bv) ----------------
        # v[t, f] = sum_e xT[e, t] * WvT[e, f] ; into vaug [v|1] blocks
        bvb_v = bvb_sb.rearrange("p (h d) -> p h d", d=D)
        for jb in range(2):
            js = slice(jb * 512, (jb + 1) * 512)
            wvh = wv_pool.tile([P, EC, 512], BF, tag="wvh")
            for ec in range(EC):
                nc.sync.dma_start(wvh[:, ec, :], wvt_d[ec, :, js])
            for kc in range(KC):
                ps = pmm.tile([P, 512], F32, tag="mm")
                for ec in range(EC):
                    nc.tensor.matmul(
                        ps[:], lhsT=xt_sb[:, ec, kc * P:(kc + 1) * P],
                        rhs=wvh[:, ec, :],
                        start=(ec == 0), stop=(ec == EC - 1),
                    )
                nc.vector.tensor_tensor(
                    vaug_h[:, kc, jb * 8:(jb + 1) * 8, 0:64],
                    ps.rearrange("p (h d) -> p h d", d=D),
                    bvb_v[:, jb * 8:(jb + 1) * 8, :], ALU.add,
                )

        # ---------------- attention + output projection ----------------
        NQB = QT // 512  # 2 query blocks
        for qtb in range(NQB):
            qs = slice(qtb * 512, (qtb + 1) * 512)
            if qtb == 0:
                for ec in range(EC):
                    nc.sync.dma_start(wot_sb[:, ec, :], wot_d[ec])
                nc.sync.dma_start(bob_sb[:], bob_d)
            for hp in range(HPAIR):
                hA, hB = 2 * hp, 2 * hp + 1
                ehA = exp_pool.tile([P, KC, 512], BF, tag="exp")
                ehB = exp_pool.tile([P, KC, 512], BF, tag="exp")
                # ---- scores^T + exp, SC_G kc-chunks per ACT ----
                for kg in range(KC // SC_G):
                    psA = psc.tile([P, SC_G, 512], F32, tag="scA")
                    psB = psc.tile([P, SC_G, 512], F32, tag="scB")
                    for i in range(SC_G):
                        kc = SC_G * kg + i
                        kslc = slice(kc * P, (kc + 1) * P)
                        tpA = dict(tile_position=(0, 0)) if USE_TILE_POS else {}
                        tpB = dict(tile_position=(64, 0)) if USE_TILE_POS else {}
                        nc.tensor.matmul(
                            psA[:, i, :], lhsT=kt_sb[0:64, hp, kslc],
                            rhs=qt_sb[0:64, hp, qs],
                            start=True, stop=True, **tpA,
                        )
                        nc.tensor.matmul(
                            psB[:, i, :], lhsT=kt_sb[64:128, hp, kslc],
                            rhs=qt_sb[64:128, hp, qs],
                            start=True, stop=True, **tpB,
                        )
                    g = slice(SC_G * kg, SC_G * (kg + 1))
                    nc.scalar.activation(ehA[:, g, :], psA[:], AF.Exp,
                                         scale=0.125)
                    nc.scalar.activation(ehB[:, g, :], psB[:], AF.Exp,
                                         scale=0.125)

                # ---- PV (+ fused denominator row at partition 64) ----
                poA = ppv.tile([P, 512], F32, tag="pvA")
                poB = ppv.tile([P, 512], F32, tag="pvB")
                for kc in range(KC):
                    nc.tensor.matmul(
                        poA[0:65, :], lhsT=vaug_h[:, kc, hA, :],
                        rhs=ehA[:, kc, :],
                        start=(kc == 0), stop=(kc == KC - 1),
                    )
                for kc in range(KC):
                    nc.tensor.matmul(
                        poB[0:65, :], lhsT=vaug_h[:, kc, hB, :],
                        rhs=ehB[:, kc, :],
                        start=(kc == 0), stop=(kc == KC - 1),
                    )

                # ---- denominators -> broadcast via K=1 fp32 matmul ----
                # srb row 64 stages the two sums rows; partitions 0:64 hold
                # the broadcast reciprocals (disjoint partition ranges).
                srb = small.tile([P, 1024], F32, tag="srb")
                nc.vector.tensor_copy(srb[64:65, 0:512], poA[64:65, :])
                nc.vector.tensor_copy(srb[64:65, 512:1024], poB[64:65, :])
                psRA = pmm.tile([P, 512], F32, tag="mm")
                nc.tensor.matmul(psRA[0:64, :], lhsT=ones_sb[64:65, :],
                                 rhs=srb[64:65, 0:512],
                                 start=True, stop=True)
                psRB = pmm.tile([P, 512], F32, tag="mm")
                nc.tensor.matmul(psRB[0:64, :], lhsT=ones_sb[64:65, :],
                                 rhs=srb[64:65, 512:1024],
                                 start=True, stop=True)
                nc.vector.reciprocal(srb[0:64, 0:512], psRA[0:64, :])
                nc.vector.reciprocal(srb[0:64, 512:1024], psRB[0:64, :])

                # ---- normalize + write outT ----
                nc.vector.tensor_tensor(outT[0:64, hp, qs], poA[0:64, :],
                                        srb[0:64, 0:512], ALU.mult)
                ot = otmp_pool.tile([P, 512], BF, tag="ot")
                nc.vector.tensor_tensor(ot[0:64, :], poB[0:64, :],
                                        srb[0:64, 512:1024], ALU.mult)
                nc.sync.dma_start(outT[64:128, hp, qs], ot[0:64, :])

            # ---- output projection for this query block ----
            for tcl in range(4):
                tc_ = qtb * 4 + tcl
                for jb in range(2):
                    fin = fin_pool.tile([P, 512], F32, tag="fin")
                    psF = pmm.tile([P, 512], F32, tag="mm")
                    for fc in range(FC):
                        nc.tensor.matmul(
                            psF[:],
                            lhsT=outT[:, fc, tc_ * P:(tc_ + 1) * P],
                            rhs=wot_sb[:, fc, jb * 512:(jb + 1) * 512],
                            start=(fc == 0), stop=(fc == FC - 1),
                        )
                    nc.vector.tensor_tensor(
                        fin[:], psF[:],
                        bob_sb[:, jb * 512:(jb + 1) * 512], ALU.add,
                    )
                    nc.sync.dma_start(out_d[tc_, :, jb * 512:(jb + 1) * 512],
                                      fin[:])

    nc.compile()
    return nc


_NC = None


def _get_nc():
    global _NC
    if _NC is None:
        _NC = build_program()
    return _NC


def _prep_core_inputs(x, Wq, bq, Wk, bk, Wv, bv, Wo, bo):
    """Build the 8 per-core input dicts (host-side sharding)."""
    bf = ml_dtypes.bfloat16
    x = np.asarray(x, dtype=np.float32)
    wqt = np.ascontiguousarray(np.asarray(Wq).T).astype(bf).reshape(EC, P, E)
    wkt = np.ascontiguousarray(np.asarray(Wk).T).astype(bf).reshape(EC, P, E)
    wvt = np.ascontiguousarray(np.asarray(Wv).T).astype(bf).reshape(EC, P, E)
    wot = np.ascontiguousarray(np.asarray(Wo).T).astype(bf).reshape(FC, P, E)
    bq_a = np.ascontiguousarray(bq, dtype=np.float32).reshape(FC, P)
    bk_a = np.ascontiguousarray(bk, dtype=np.float32).reshape(FC, P)
    bvb = np.ascontiguousarray(
        np.broadcast_to(np.asarray(bv, np.float32)[None, :], (P, E)))
    bob = np.ascontiguousarray(
        np.broadcast_to(np.asarray(bo, np.float32)[None, :], (P, E)))

    in_maps = []
    for c in range(NCORES):
        b, qh = c // 2, c % 2
        xb = x[b]  # [T, E]
        own = xb[qh * QT:(qh + 1) * QT]
        other = xb[(1 - qh) * QT:(2 - qh) * QT]
        xperm = np.concatenate([own, other], axis=0)      # [T, E]
        xt = np.ascontiguousarray(xperm.T).astype(bf).reshape(EC, P, T)
        in_maps.append({
            "xt": xt, "wqt": wqt, "wkt": wkt, "wvt": wvt, "wot": wot,
            "bq": bq_a, "bk": bk_a, "bvb": bvb, "bob": bob,
        })
    return in_maps


def kernel(x, Wq, bq, Wk, bk, Wv, bv, Wo, bo):
    nc = _get_nc()
    in_maps = _prep_core_inputs(x, Wq, bq, Wk, bk, Wv, bv, Wo, bo)
    res = run_bass_kernel_spmd(nc, in_maps, list(range(NCORES)))
    out = np.empty((B, T, E), np.float32)
    for c in range(NCORES):
        b, qh = c // 2, c % 2
        out[b, qh * QT:(qh + 1) * QT] = res.results[c]["out"].reshape(QT, E)
    return out
